# revision 33
# baseline (speedup 1.0000x reference)
"""CPC NCE loss kernel for Trainium2, 8 NeuronCores.

Sharding: the 224 independent (i,k,j) NCE combos are split 28 per core.
Per core the 28 combos form 7 "units" (one (i,k) pair restricted to 4
consecutive j positions = 256 rows) of 2 "chunks" (128 rows) each.

Per chunk (128 rows r = (j, b)):
  zh^T = Wk^T.T @ C^T     (PE bf16, f32 PSUM; bias + fp8e4m3 cast on ACT)
  raw  = zh8 @ Zneg8      (PE fp8e4m3 perf_mode=DoubleRow, K=256 per
                           call: the shared (512, 4096) negatives matrix
                           is packed [ki, ko=2, n] so two 128-feature
                           chunks contract per call; n = (h*8+w)*64 + b
                           so the self-batch mask is the same diagonal
                           pattern for every block)
  pos  = diag(zh8 @ Zpos8^T)  (PE normal-mode fp8 + DVE eye-masked sum)
  esc  = exp(raw - M)     (ACT straight out of PSUM -> bf16 SBUF; M a
                           constant shift - the log-sum-exp is shift
                           invariant and scores are ~[-60, 60])
  S    = sum esc*selfmask (DVE scalar_tensor_tensor w/ accumulator;
                           selfmask zeroes the 64 self-batch columns)
Loss tail nce = (pos - M) - log(exp(pos - M) + S) is emitted for chunks
0..11 while unit 6 still streams, and for 12..13 at the end.
Host sums the 8 cores' (128, 14) nce tiles and takes -mean.

The per-unit linear layer is software-pipelined between the two chunks
of the previous unit, unit inputs are prefetched 2+ units ahead, and a
short zero-matmul warmup keeps the PE busy during the initial DMA wait
so the HAM clock gate opens before real work starts.
"""

import numpy as np
import ml_dtypes

import concourse.bass as bass
import concourse.tile as tile
from concourse import mybir
from concourse.vector_clock import ScopedClock
from concourse.bass_utils import run_bass_kernel_spmd

B, D, H, W = 64, 512, 8, 8
NCORES = 8
NUNITS = 7            # units per core
NCHUNKS = 2 * NUNITS  # chunks per core
NG = 4                # 1024-wide negative groups per chunk
EC = 4                # 128-wide feature chunks
BF16 = ml_dtypes.bfloat16
FP8 = ml_dtypes.float8_e4m3
M_SHIFT = 45.0

F32 = mybir.dt.float32
BF = mybir.dt.bfloat16
F8 = mybir.dt.float8e4

LAST_RESULTS = None  # BassKernelResults of the most recent run (for test.py)

_cache = {}


def _fix_ldweights_waits(nc):
    """walrus's redundant-LDW optimization (active for perf-mode matmuls)
    rejects Ldweights carrying sem waits; hoist them onto PE NOPs."""
    k = 0
    for f in nc.m.functions:
        for bb in f.blocks:
            newlist = []
            changed = False
            for inst in bb.instructions:
                si = inst.sync_info
                if (type(inst).__name__ == "InstLdweights"
                        and si is not None and si.on_wait):
                    nop = mybir.InstNoOp(name=f"I-ldwwait-{k}", ins=[], outs=[])
                    k += 1
                    nop.engine = inst.engine
                    nop.sync_info = mybir.SyncInfo(
                        on_wait=list(si.on_wait), on_update=[]
                    )
                    newlist.append(nop)
                    inst.sync_info = mybir.SyncInfo(
                        on_wait=[], on_update=list(si.on_update or [])
                    )
                    changed = True
                newlist.append(inst)
            if changed:
                bb.instructions = newlist


def _split_multi_waits(nc):
    """walrus in this container accepts at most ONE sync wait per
    instruction; hoist extra waits onto preceding same-engine NOPs."""
    k = 0
    for f in nc.m.functions:
        for bb in f.blocks:
            newlist = []
            changed = False
            for inst in bb.instructions:
                si = inst.sync_info
                if si is not None and si.on_wait and len(si.on_wait) > 1:
                    waits = list(si.on_wait)
                    for w in waits[:-1]:
                        nop = mybir.InstNoOp(name=f"I-wsplit-{k}", ins=[], outs=[])
                        k += 1
                        nop.engine = inst.engine
                        nop.sync_info = mybir.SyncInfo(on_wait=[w], on_update=[])
                        newlist.append(nop)
                    inst.sync_info = mybir.SyncInfo(
                        on_wait=[waits[-1]], on_update=list(si.on_update or [])
                    )
                    changed = True
                newlist.append(inst)
            if changed:
                bb.instructions = newlist


class _TileContext(tile.TileContext):
    """Tail drain variant that keeps <=1 sem wait per instruction."""

    def _drain_and_barrier(self, tick_clock, wait_clock):
        nc = self.nc
        probe = nc.sync.nop(nofuse=True)
        wait_clock.add_sem_waits(
            probe.ins, ScopedClock({None: tick_clock.global_clock})
        )
        si = probe.ins.sync_info
        if si is not None and si.on_wait and len(si.on_wait) > 1:
            waits = list(si.on_wait)
            probe.ins.sync_info = mybir.SyncInfo(
                on_wait=waits[:1], on_update=list(si.on_update or [])
            )
            for w in waits[1:]:
                n2 = nc.sync.nop(nofuse=True)
                n2.ins.sync_info = mybir.SyncInfo(on_wait=[w], on_update=[])
        nc.sync.drain()
        nc.all_engine_barrier()
        assert self.sems is not None
        popped = nc._tile_sem_poison_stack.pop()
        assert popped is self._sem_poison
        nc.clear_and_free_semaphores(list(self.sems.allocated().values()))


def _build_module():
    nc = bass.Bass("TRN2", target_bir_lowering=False, debug=False)
    ap = {}
    ap["zn"] = nc.dram_tensor("zn", [NG, 128, EC, 1024], F8, kind="ExternalInput").ap()
    ap["wcc"] = nc.dram_tensor("wcc", [NUNITS, 128, EC, 768], BF, kind="ExternalInput").ap()
    ap["zpc"] = nc.dram_tensor("zpc", [NUNITS, 128, 2, EC, 128], F8, kind="ExternalInput").ap()
    ap["bgc"] = nc.dram_tensor("bgc", [NUNITS, 128, EC], F32, kind="ExternalInput").ap()
    ap["selfm"] = nc.dram_tensor("selfm", [128, 4, 1024], BF, kind="ExternalInput").ap()
    ap["eye"] = nc.dram_tensor("eye", [128, 128], F32, kind="ExternalInput").ap()
    out_ap = nc.dram_tensor("out", [128, NCHUNKS], F32, kind="ExternalOutput").ap()

    Exp = mybir.ActivationFunctionType.Exp
    Ln = mybir.ActivationFunctionType.Ln
    Ident = mybir.ActivationFunctionType.Identity
    Add = mybir.AluOpType.add
    Mult = mybir.AluOpType.mult
    Sub = mybir.AluOpType.subtract
    X = mybir.AxisListType.X
    DR = mybir.MatmulPerfMode.DoubleRow

    with _TileContext(nc) as tc:
        with (
            tc.tile_pool(name="consts", bufs=1) as consts,
            tc.tile_pool(name="wpool", bufs=3) as wpool,
            tc.tile_pool(name="zhpool", bufs=3) as zhpool,
            tc.tile_pool(name="zppool", bufs=5) as zppool,
            tc.tile_pool(name="bgpool", bufs=3) as bgpool,
            tc.tile_pool(name="escpool", bufs=6) as escpool,
            tc.tile_pool(name="scrpool", bufs=5) as scrpool,
            tc.tile_pool(name="dscpool", bufs=2) as dscpool,
            tc.tile_pool(name="ps_raw", bufs=3, space="PSUM") as ps_raw,
            tc.tile_pool(name="ps_zha", bufs=1, space="PSUM") as ps_zha,
            tc.tile_pool(name="ps_zhb", bufs=1, space="PSUM") as ps_zhb,
        ):
            def load_unit(u):
                wcs = []
                for dc in range(EC):
                    w1 = wpool.tile([128, 768], BF, name=f"wc{dc}")
                    nc.sync.dma_start(w1[:], ap["wcc"][u, :, dc])
                    wcs.append(w1)
                bg = bgpool.tile([128, EC], F32)
                nc.sync.dma_start(bg[:], ap["bgc"][u])
                return wcs, bg

            def load_zp(u):
                zp = zppool.tile([128, 2, EC, 128], F8)
                nc.sync.dma_start(zp[:], ap["zpc"][u])
                return zp

            def mm1(wcs, bg):
                """zh^T[e, r] for a unit's 256 rows, bias-added, cast to
                fp8e4m3 on ACT. Separate half tiles so half 1 never waits
                on half 0's casts."""
                zh = zhpool.tile([128, EC, 256], F8)
                for half in range(2):
                    zh_ps = (ps_zha if half == 0 else ps_zhb).tile(
                        [128, 2, 256], F32, name="zh_ps")
                    for e2 in range(2):
                        ec = 2 * half + e2
                        for dc in range(EC):
                            nc.tensor.matmul(
                                zh_ps[:, e2, :],
                                wcs[dc][:, ec * 128:(ec + 1) * 128],
                                wcs[dc][:, 512:768],
                                start=(dc == 0),
                                stop=(dc == EC - 1),
                            )
                    for e2 in range(2):
                        ec = 2 * half + e2
                        nc.scalar.activation(
                            zh[:, ec, :], zh_ps[:, e2, :], Ident,
                            bias=bg[:, ec:ec + 1], scale=1.0,
                        )
                return zh

            warm = consts.tile([128, 512], BF)
            nc.vector.memset(warm[:], 0.0)
            # DMA issue order is the startup critical path: wcc planes for
            # unit 0 first, then the two zn groups the first chunk touches
            wcs0 = [wpool.tile([128, 768], BF, name=f"wc{dc}")
                    for dc in range(EC)]
            zn_ts = [consts.tile([128, EC, 1024], F8, name=f"zn{g}")
                     for g in range(NG)]
            nc.sync.dma_start(wcs0[0][:], ap["wcc"][0, :, 0])
            nc.sync.dma_start(wcs0[1][:], ap["wcc"][0, :, 1])
            nc.sync.dma_start(zn_ts[0][:], ap["zn"][0])
            nc.sync.dma_start(wcs0[2][:], ap["wcc"][0, :, 2])
            nc.sync.dma_start(wcs0[3][:], ap["wcc"][0, :, 3])
            nc.sync.dma_start(zn_ts[1][:], ap["zn"][1])
            bg0 = bgpool.tile([128, EC], F32, name="bg")
            nc.sync.dma_start(bg0[:], ap["bgc"][0])
            u0 = (wcs0, bg0)
            nc.sync.dma_start(zn_ts[2][:], ap["zn"][2])
            nc.sync.dma_start(zn_ts[3][:], ap["zn"][3])
            selfm4_t = consts.tile([128, 4, 1024], BF)
            eye_t = consts.tile([128, 128], F32)
            zps = load_zp(0)
            nc.sync.dma_start(selfm4_t[:], ap["selfm"][:])
            nc.sync.dma_start(eye_t[:], ap["eye"][:])
            u1 = load_unit(1)
            out_t = consts.tile([128, NCHUNKS], F32)
            poscol = consts.tile([128, NCHUNKS], F32)
            scol = consts.tile([128, NCHUNKS], F32)
            s_all = consts.tile([128, NCHUNKS, 4], F32)
            nc.vector.memset(s_all[:], 0.0)
            e14 = consts.tile([128, NCHUNKS], F32)
            t14 = consts.tile([128, NCHUNKS], F32)
            l14 = consts.tile([128, NCHUNKS], F32)

            def emit_tail(ts):
                # nce tail: out = (pos - M) - log(exp(pos - M) + S)
                nc.vector.reduce_sum(
                    out=scol[:, ts], in_=s_all[:, ts], axis=X,
                )
                nc.scalar.activation(
                    e14[:, ts], poscol[:, ts], Exp, bias=negM[:, 0:1],
                )
                nc.vector.tensor_add(t14[:, ts], e14[:, ts], scol[:, ts])
                nc.scalar.activation(l14[:, ts], t14[:, ts], Ln)
                nc.vector.scalar_tensor_tensor(
                    out=out_t[:, ts], in0=poscol[:, ts], scalar=-M_SHIFT,
                    in1=l14[:, ts], op0=Add, op1=Sub,
                )
                nc.sync.dma_start(out_ap[:, ts], out_t[:, ts])
            negM = consts.tile([128, 1], F32)
            nc.vector.memset(negM[:], -M_SHIFT)

            # keep the PE busy while the first input DMAs land so HAM
            # un-throttles to 2.4 GHz; sized to fit inside the DMA wait
            for w in range(8):
                wp = ps_zhb.tile([128, 2, 256], F32, name="zh_ps")
                nc.tensor.matmul(wp[:, 0, :], warm[:, 0:128], warm[:, 0:256],
                                 start=True, stop=True)

            pending = u1
            zh = mm1(*u0)
            for u in range(NUNITS):
                zh_next = None
                zps_next = None
                for h_ in range(2):
                    t_idx = 2 * u + h_
                    rs = slice(h_ * 128, (h_ + 1) * 128)

                    for gp in ((0, 1), (2, 3)):
                        rt = {g: ps_raw.tile([128, 1024], F32, name="raw_ps")
                              for g in gp}
                        for h in range(2):
                            for g in gp:
                                for q in range(2):
                                    nc.tensor.matmul(
                                        rt[g][:, q * 512:(q + 1) * 512],
                                        zh[:, 2 * h:2 * h + 2, rs],
                                        zn_ts[g][:, 2 * h:2 * h + 2,
                                                 q * 512:(q + 1) * 512],
                                        start=(h == 0),
                                        stop=(h == 1),
                                        perf_mode=DR,
                                    )
                        for g in gp:
                            esc = escpool.tile([128, 1024], BF)
                            nc.scalar.activation(
                                esc[:], rt[g][:], Exp,
                                bias=negM[:, 0:1], scale=1.0,
                            )
                            scr = scrpool.tile([128, 1024], BF)
                            nc.vector.scalar_tensor_tensor(
                                out=scr[:], in0=esc[:], scalar=1.0,
                                in1=selfm4_t[:, 0], op0=Mult, op1=Mult,
                                accum_out=s_all[:, t_idx, g:g + 1],
                            )

                    # positives: diag(zh_chunk @ Zpos^T); normal-mode fp8
                    # so the 128-col FWL weight loads hide under the streams
                    pos_ps = ps_raw.tile([128, 1024], F32, name="raw_ps")
                    for e in range(EC):
                        nc.tensor.matmul(
                            pos_ps[:, 0:128], zh[:, e, rs],
                            zps[:, h_, e, :],
                            start=(e == 0), stop=(e == EC - 1),
                        )
                    dsc = dscpool.tile([128, 128], BF)
                    nc.vector.scalar_tensor_tensor(
                        out=dsc[:], in0=pos_ps[:, 0:128], scalar=1.0,
                        in1=eye_t[:], op0=Mult, op1=Mult,
                        accum_out=poscol[:, t_idx:t_idx + 1],
                    )

                    if u == NUNITS - 1 and h_ == 0:
                        # chunk 12 done: emit its tail now so only chunk 13
                        # drains after the last matmul
                        emit_tail(slice(NCHUNKS - 2, NCHUNKS - 1))
                    # pipeline the next unit's linear layer between chunks
                    if h_ == 0 and u + 1 < NUNITS:
                        zps_next = load_zp(u + 1)
                        zh_next = mm1(*pending)
                        if u + 2 < NUNITS:
                            pending = load_unit(u + 2)
                zh = zh_next
                zps = zps_next
                if u == NUNITS - 2:
                    # chunks 0..11 are done: emit their loss tail early so
                    # only the last unit's 2 chunks drain after the compute
                    emit_tail(slice(0, 2 * NUNITS - 2))

            emit_tail(slice(NCHUNKS - 1, NCHUNKS))

    _fix_ldweights_waits(nc)
    _split_multi_waits(nc)
    return nc


def _prep_inputs(Z, C, Wk, bk):
    """Host-side layout prep + per-core slicing (partition-major so every
    SBUF tile loads with a single contiguous DMA)."""
    ii, kk = np.triu_indices(H, 1)
    # negatives (512, 4096) with n=(h*8+w)*64+b, packed [NG, 128, EC, 1024]
    zn = (
        Z.transpose(1, 2, 3, 0).reshape(EC, 128, NG, 1024)
        .transpose(2, 1, 0, 3)
    )
    zn = np.ascontiguousarray(zn).astype(FP8)
    WkT = Wk.transpose(0, 2, 1).reshape(7, EC, 128, 512).transpose(0, 2, 1, 3)
    WkT = np.ascontiguousarray(WkT).astype(BF16)  # (7, 128, 4, 512)
    Ctr = np.ascontiguousarray(C.transpose(2, 1, 3, 0))  # (H, D, W, B)
    Ztr = np.ascontiguousarray(Z.transpose(2, 1, 3, 0))  # (H, D, W, B)

    rr = np.arange(128)
    selfm1 = np.where(
        (np.arange(1024)[None, :] % 64) == (rr[:, None] % 64),
        np.float32(0.0), np.float32(1.0),
    ).astype(BF16)
    selfm = np.stack([selfm1] * 4, axis=1)  # (128, 4, 1024)
    eye = np.eye(128, dtype=np.float32)

    in_maps = []
    for c in range(NCORES):
        wcc = np.empty((NUNITS, 128, EC, 768), BF16)
        zpc = np.empty((NUNITS, 128, 2, EC, 128), FP8)
        bgc = np.empty((NUNITS, 128, EC), np.float32)
        for u in range(NUNITS):
            g = NUNITS * c + u
            p = g // 2
            w0 = 4 * (g % 2)
            i_, k_ = int(ii[p]), int(kk[p])
            wcc[u, :, :, :512] = WkT[k_ - 1]
            wcc[u, :, :, 512:768] = (
                Ctr[i_][:, w0:w0 + 4, :].reshape(EC, 128, 256)
                .transpose(1, 0, 2).astype(BF16)
            )
            bgc[u] = bk[k_ - 1].reshape(EC, 128).T
            for h_ in range(2):
                wp0 = w0 + 2 * h_
                zpc[u, :, h_] = (
                    Ztr[k_][:, wp0:wp0 + 2, :].reshape(EC, 128, 128)
                    .transpose(1, 0, 2).astype(FP8)
                )
        in_maps.append({
            "zn": zn, "wcc": wcc, "zpc": zpc, "bgc": bgc,
            "selfm": selfm, "eye": eye,
        })
    return in_maps


def kernel(Z, C, Wk, bk):
    global LAST_RESULTS
    Z = np.asarray(Z, np.float32)
    C = np.asarray(C, np.float32)
    Wk = np.asarray(Wk, np.float32)
    bk = np.asarray(bk, np.float32)

    if "nc" not in _cache:
        _cache["nc"] = _build_module()
    nc = _cache["nc"]

    in_maps = _prep_inputs(Z, C, Wk, bk)
    res = run_bass_kernel_spmd(nc, in_maps, core_ids=list(range(NCORES)))
    LAST_RESULTS = res
    total = np.float64(0.0)
    for c in range(NCORES):
        total += np.sum(res.results[c]["out"].astype(np.float64))
    loss = -(total / (NCORES * NCHUNKS * 128))
    return np.array(loss, dtype=np.float32)
